# revision 7
# baseline (speedup 1.0000x reference)
"""Trainium2 Bass kernel for nn_Attention_83081847374268 (sparse sliding-window GQA).

Sharding: 8 cores = batch (2, data parallel) x kv-head (4, tensor parallel).
Each core computes, for its (b, kh): q/k/v projections (2 q heads, 1 kv head),
QK-RMSNorm + RoPE, banded sliding-window attention, and a partial output
projection against its 512-row slice of wout.  The host sums the 4 partials
per batch (the TP reduction) and stacks the batches.

Device dataflow (per core):
  stage A: stream xT column-chunks; matmul projections directly in transposed
           layout (qT/kT [head_dim, T]); RMSNorm via ones-matmul variance +
           PE-broadcast rstd; RoPE fused with the rstd multiply on DVE.
           v computed in natural layout [T, 256] and bounced via DRAM.
  stage B: per 128-query tile: S = qT.T @ kT over a host-chosen key window,
           additive mask bias (host-precomputed, handles any attn_mask /
           positions pattern), exp on ACT with fused row-sum, normalize on
           DVE, PE-transpose P, PV matmuls accumulate encoded^T.
  stage C: out partial = encT.T @ wout_slice, DMA per 128-row tile.

All matmuls run as float32r (TRN2 rounds fp32r operands to 12 mantissa bits
on write; full speed at moving-dim >= 256).
"""
import sys

sys.path.insert(0, "/opt/trn_rl_repo")

import numpy as np
import ml_dtypes

import concourse.bacc as bacc
import concourse.mybir as mybir
from concourse.bass_utils import run_bass_kernel_spmd
from concourse.tile import TileContext
from concourse.alu_op_type import AluOpType

F32 = mybir.dt.float32
F32R = mybir.dt.float32r
BF16 = mybir.dt.bfloat16
ACTF = mybir.ActivationFunctionType

B, T, WIDTH = 2, 2048, 2048
NUM_HEADS, NUM_KV_HEADS, HEAD_DIM = 8, 4, 256
GROUPS = NUM_HEADS // NUM_KV_HEADS  # 2 q heads per kv head (= per core)
WINDOW = 512
ROPE_BASE = 10000.0
ALPHA = HEAD_DIM ** -0.5
MASK_NEG = -100.0  # exp(S/16 + MASK_NEG) == 0 for |S|<=~16; exact in bf16

NT = T // 128           # 16 query tiles
TCH = 256               # stage-A t-chunk width
NTCH = T // TCH         # 8
NW = WIDTH // 128       # 16 contraction chunks

_prog_cache = {}
DEBUG_TAPS = False


def _round_up(x, m):
    return (x + m - 1) // m * m


def _geometry(positions, attn_mask):
    """Per-query-tile key windows from the actual mask/positions data."""
    pos = np.asarray(positions)
    am = np.asarray(attn_mask)
    pd = pos[:, :, None].astype(np.int64) - pos[:, None, :].astype(np.int64)
    valid = am & (np.abs(pd) < WINDOW)  # [B, T, T] bool
    assert valid.any(axis=2).all(), "a query row with no valid key is unsupported"
    js = []
    wmax = 0
    for it in range(NT):
        cols = valid[:, it * 128:(it + 1) * 128, :].any(axis=(0, 1))
        idx = np.nonzero(cols)[0]
        j_lo, j_hi = int(idx[0]), int(idx[-1]) + 1
        j0 = (j_lo // 128) * 128
        wmax = max(wmax, j_hi - j0)
        js.append(j0)
    Wb = max(256, _round_up(wmax, 256))
    Wb = min(Wb, T)
    js = tuple(max(0, min(j, T - Wb)) for j in js)
    return valid, Wb, js


def _pieces(Wb):
    """Split Wb into matmul moving-dim pieces of 512/256 (keeps fp32r fast)."""
    out = []
    rem = Wb
    while rem > 512:
        out.append(512)
        rem -= 512
    if rem:
        out.append(rem)
    return out


def _rope_tables(pos_b, scale):
    """cos/sin tables in [head_dim/2, T] (transposed) layout, gain folded in."""
    d = np.arange(HEAD_DIM // 2, dtype=np.float32)
    timescale = (ROPE_BASE ** (2.0 / HEAD_DIM * d)).astype(np.float32)
    rad = pos_b.astype(np.float32)[None, :] / timescale[:, None]  # [128, T]
    cos, sin = np.cos(rad).astype(np.float32), np.sin(rad).astype(np.float32)
    g1 = (1.0 + scale[:HEAD_DIM // 2]).astype(np.float32)[:, None]
    g2 = (1.0 + scale[HEAD_DIM // 2:]).astype(np.float32)[:, None]
    # o1 = a1*C1 - a2*S2 ; o2 = a2*C2 + a1*S1
    return (cos * g1, sin * g1, cos * g2, sin * g2)  # C1, S1, C2, S2


def _build(Wb, js, shared_tables, debug_taps=False):
    nc = bacc.Bacc("TRN2", target_bir_lowering=False, debug=False, num_devices=8)

    def din(name, shape, dt):
        return nc.dram_tensor(name, shape, dt, kind="ExternalInput").ap()

    xT = din("xT", [WIDTH, T], F32R)
    wq = din("wq", [WIDTH, 512], F32R)
    wk = din("wk", [WIDTH, 256], F32R)
    wv = din("wv", [WIDTH, 256], F32R)
    wout = din("wout", [512, T], F32R)
    ident_d = din("ident", [128, 128], F32R)
    ones1_d = din("ones1", [1, 128], F32R)    # K=1 broadcast lhsT
    onesc_d = din("onesc", [128, 1], F32R)    # partition-sum lhsT
    bias_d = din("bias", [NT, 128, Wb], BF16)
    tab_names = ["ct", "st"] if shared_tables else [
        "cq1", "sq1", "cq2", "sq2", "ck1", "sk1", "ck2", "sk2"]
    tabs = {n: din(n, [128, T], F32) for n in tab_names}
    yp = nc.dram_tensor("yp", [T, T], F32, kind="ExternalOutput").ap()
    v_dram = nc.dram_tensor("v_scratch", [T, 256], F32R).ap()

    taps = {}
    if debug_taps:
        for nm, shape in (("qT_tap", [512, T]), ("kT_tap", [256, T]),
                          ("v_tap", [T, 256]), ("encT_tap", [512, T])):
            taps[nm] = nc.dram_tensor(nm, shape, F32, kind="ExternalOutput").ap()

    pieces = _pieces(Wb)
    NJ = Wb // 128  # P-transpose blocks per tile
    # per-group (2 query tiles) union of key chunks, and which halves exist
    groups = []
    for g in range(NT // 2):
        w0 = set(range(js[2 * g] // 128, js[2 * g] // 128 + NJ))
        w1 = set(range(js[2 * g + 1] // 128, js[2 * g + 1] // 128 + NJ))
        groups.append([(jc, jc in w0, jc in w1) for jc in sorted(w0 | w1)])

    with TileContext(nc) as tc:
        with (
            tc.tile_pool(name="persist", bufs=1) as pp,
            tc.tile_pool(name="qk_store", bufs=1) as qkp,
        ):
            ident = pp.tile([128, 128], F32R)
            nc.sync.dma_start(out=ident[:], in_=ident_d[:])
            ones1 = pp.tile([1, 128], F32R)
            nc.sync.dma_start(out=ones1[:], in_=ones1_d[:])
            onesc = pp.tile([128, 1], F32R)
            nc.sync.dma_start(out=onesc[:], in_=onesc_d[:])
            epsb = pp.tile([1, 1], F32)
            nc.any.memset(epsb[:], 1e-6)
            zrow_f = pp.tile([128, 128], F32)
            nc.any.memset(zrow_f[:], 0.0)
            zero_r = pp.tile([128, 128], F32R)
            nc.vector.tensor_copy(zero_r[:], zrow_f[:])

            qT = [qkp.tile([128, T], F32R, tag=f"qT{c}", name=f"qT{c}") for c in range(4)]
            kT = [qkp.tile([128, T], F32R, tag=f"kT{c}", name=f"kT{c}") for c in range(2)]

            # ---------------- stage A: projections + RMSNorm + RoPE ----------
            with (
                tc.tile_pool(name="wpool", bufs=1) as wp,
                tc.tile_pool(name="xpool", bufs=2) as xp,
                tc.tile_pool(name="tabpool", bufs=2) as tp,
                tc.tile_pool(name="sa", bufs=2) as sa,
                tc.tile_pool(name="psA", bufs=2, space="PSUM") as psA,
                tc.tile_pool(name="psA1", bufs=1, space="PSUM") as psA1,
            ):
                wq_t = wp.tile([128, NW * 512], F32R)
                wk_t = wp.tile([128, NW * 256], F32R)
                wv_t = wp.tile([128, NW * 256], F32R)
                wq_r = wq.rearrange("(c p) m -> c p m", p=128)
                wk_r = wk.rearrange("(c p) m -> c p m", p=128)
                wv_r = wv.rearrange("(c p) m -> c p m", p=128)
                for c in range(NW):
                    nc.sync.dma_start(out=wq_t[:, c * 512:(c + 1) * 512], in_=wq_r[c])
                    nc.sync.dma_start(out=wk_t[:, c * 256:(c + 1) * 256], in_=wk_r[c])
                    nc.sync.dma_start(out=wv_t[:, c * 256:(c + 1) * 256], in_=wv_r[c])

                xT_r = xT.rearrange("(c p) t -> c p t", p=128)
                if shared_tables:
                    q_tabs = k_tabs = ("ct", "st", "ct", "st")
                else:
                    q_tabs = ("cq1", "sq1", "cq2", "sq2")
                    k_tabs = ("ck1", "sk1", "ck2", "sk2")
                units = [
                    (wq_t, 512, 0, q_tabs, qT, 0),
                    (wq_t, 512, 256, q_tabs, qT, 2),
                    (wk_t, 256, 0, k_tabs, kT, 0),
                ]
                for tci in range(NTCH):
                    t0 = tci * TCH
                    xts = xp.tile([128, NW * TCH], F32R, tag="xts")
                    for wc in range(NW):
                        nc.sync.dma_start(
                            out=xts[:, wc * TCH:(wc + 1) * TCH],
                            in_=xT_r[wc][:, t0:t0 + TCH],
                        )
                    # v projection (natural layout), bounce to DRAM
                    for s in range(TCH // 128):
                        psv = psA.tile([128, 256], F32, tag="psv")
                        for wc in range(NW):
                            nc.tensor.matmul(
                                psv[:],
                                xts[:, wc * TCH + s * 128: wc * TCH + (s + 1) * 128],
                                wv_t[:, wc * 256:(wc + 1) * 256],
                                start=(wc == 0), stop=(wc == NW - 1),
                            )
                        vsb = sa.tile([128, 256], F32R, tag="vsb")
                        nc.scalar.activation(vsb[:], psv[:], ACTF.Copy)
                        nc.sync.dma_start(
                            out=v_dram[t0 + s * 128: t0 + (s + 1) * 128, :], in_=vsb[:])
                        if debug_taps:
                            nc.sync.dma_start(
                                out=taps["v_tap"][t0 + s * 128: t0 + (s + 1) * 128, :],
                                in_=vsb[:].bitcast(F32))
                    # q/k projections in transposed layout + norm + rope
                    tabt = {}
                    for name in dict.fromkeys(q_tabs + k_tabs):
                        tt = tp.tile([128, TCH], F32, tag=name, name=f"tab_{name}")
                        nc.sync.dma_start(out=tt[:], in_=tabs[name][:, t0:t0 + TCH])
                        tabt[name] = tt
                    for w_t, wcols, cbase, tkeys, dest, dbase in units:
                        ps1 = psA.tile([128, TCH], F32, tag="pp1")
                        ps2 = psA.tile([128, TCH], F32, tag="pp2")
                        for ps, cc in ((ps1, 0), (ps2, 1)):
                            coff = cbase + cc * 128
                            for wc in range(NW):
                                nc.tensor.matmul(
                                    ps[:],
                                    w_t[:, wc * wcols + coff: wc * wcols + coff + 128],
                                    xts[:, wc * TCH:(wc + 1) * TCH],
                                    start=(wc == 0), stop=(wc == NW - 1),
                                )
                        sq1 = sa.tile([128, TCH], F32R, tag="sq1")
                        sq2 = sa.tile([128, TCH], F32R, tag="sq2")
                        nc.scalar.activation(sq1[:], ps1[:], ACTF.Square)
                        nc.scalar.activation(sq2[:], ps2[:], ACTF.Square)
                        psvar = psA1.tile([1, TCH], F32, tag="psvar")
                        nc.tensor.matmul(psvar[:], onesc[:], sq1[:], start=True, stop=False)
                        nc.tensor.matmul(psvar[:], onesc[:], sq2[:], start=False, stop=True)
                        stdv = sa.tile([1, TCH], F32, tag="stdv")
                        nc.scalar.activation(stdv[:], psvar[:], ACTF.Sqrt,
                                             scale=1.0 / HEAD_DIM, bias=epsb[:])
                        rstd = sa.tile([1, TCH], F32R, tag="rstd")
                        with nc.allow_low_precision(reason="rstd f32r for PE broadcast"):
                            nc.vector.reciprocal(rstd[:], stdv[:])
                        psb = psA1.tile([128, TCH], F32, tag="psb")
                        nc.tensor.matmul(psb[:], ones1[:], rstd[:], start=True, stop=True)
                        rb = sa.tile([128, TCH], F32, tag="rb")
                        nc.scalar.activation(rb[:], psb[:], ACTF.Copy)
                        a1 = sa.tile([128, TCH], F32, tag="a1")
                        a2 = sa.tile([128, TCH], F32, tag="a2")
                        nc.vector.tensor_tensor(a1[:], ps1[:], rb[:], AluOpType.mult)
                        nc.vector.tensor_tensor(a2[:], ps2[:], rb[:], AluOpType.mult)
                        C1, S1, C2, S2 = (tabt[k] for k in tkeys)
                        m1 = sa.tile([128, TCH], F32, tag="m1")
                        m2 = sa.tile([128, TCH], F32, tag="m2")
                        m3 = sa.tile([128, TCH], F32, tag="m3")
                        m4 = sa.tile([128, TCH], F32, tag="m4")
                        nc.vector.tensor_tensor(m1[:], a1[:], C1[:], AluOpType.mult)
                        nc.vector.tensor_tensor(m2[:], a2[:], S2[:], AluOpType.mult)
                        nc.vector.tensor_tensor(
                            dest[dbase][:, t0:t0 + TCH], m1[:], m2[:], AluOpType.subtract)
                        nc.vector.tensor_tensor(m3[:], a2[:], C2[:], AluOpType.mult)
                        nc.vector.tensor_tensor(m4[:], a1[:], S1[:], AluOpType.mult)
                        nc.vector.tensor_tensor(
                            dest[dbase + 1][:, t0:t0 + TCH], m3[:], m4[:], AluOpType.add)
            if debug_taps:
                for c in range(4):
                    nc.sync.dma_start(out=taps["qT_tap"][c * 128:(c + 1) * 128, :],
                                      in_=qT[c][:].bitcast(F32))
                for c in range(2):
                    nc.sync.dma_start(out=taps["kT_tap"][c * 128:(c + 1) * 128, :],
                                      in_=kT[c][:].bitcast(F32))

            # ---------------- stage B: banded attention ----------------------
            with (
                tc.tile_pool(name="encp", bufs=1) as encp,
                tc.tile_pool(name="woutp", bufs=1) as woutp,
            ):
                encT = [encp.tile([128, T], F32R, tag=f"encT{c}", name=f"encT{c}") for c in range(4)]
                wout_t = [woutp.tile([128, T], F32R, tag=f"wo{c}", name=f"wo{c}") for c in range(4)]
                wout_r = wout.rearrange("(c p) t -> c p t", p=128)
                for c in range(4):
                    nc.sync.dma_start(out=wout_t[c][:], in_=wout_r[c])

                with (
                    tc.tile_pool(name="vstage", bufs=2) as vsp,
                    tc.tile_pool(name="ptp", bufs=2) as ptp,
                    tc.tile_pool(name="sb", bufs=2) as sbp,
                    tc.tile_pool(name="psS", bufs=2, space="PSUM") as psS,
                    tc.tile_pool(name="psT", bufs=2, space="PSUM") as psT,
                    tc.tile_pool(name="psE", bufs=2, space="PSUM") as psE,
                ):
                    for g in range(NT // 2):
                        ginfo = groups[g]
                        vt = {}
                        for jc, _, _ in ginfo:
                            v_t = vsp.tile([128, 256], F32R, tag=f"v{len(vt)}", name=f"vstg{len(vt)}")
                            nc.sync.dma_start(
                                out=v_t[:], in_=v_dram[jc * 128:(jc + 1) * 128, :])
                            vt[jc] = v_t
                        pts = {h: [ptp.tile([128, 256], F32R, tag=f"pt{h}_{i}", name=f"pt{h}_{i}")
                                   for i in range(len(ginfo))] for h in range(2)}
                        for half in range(2):
                            it = 2 * g + half
                            jst = js[it]
                            bias_t = sbp.tile([128, Wb], BF16, tag="bias")
                            nc.sync.dma_start(out=bias_t[:], in_=bias_d[it])
                            for h in range(2):
                                S_ps = psS.tile([128, Wb], F32, tag="S")
                                for cc in range(2):
                                    col = 0
                                    for pw in pieces:
                                        nc.tensor.matmul(
                                            S_ps[:, col:col + pw],
                                            qT[2 * h + cc][:, it * 128:(it + 1) * 128],
                                            kT[cc][:, jst + col: jst + col + pw],
                                            start=(cc == 0), stop=(cc == 1),
                                        )
                                        col += pw
                                S_b = sbp.tile([128, Wb], F32, tag="Sb")
                                nc.vector.scalar_tensor_tensor(
                                    S_b[:], S_ps[:], ALPHA, bias_t[:],
                                    AluOpType.mult, AluOpType.add)
                                P_t = sbp.tile([128, Wb], F32R, tag="P")
                                denom = sbp.tile([128, 1], F32, tag="den")
                                nc.scalar.activation(P_t[:], S_b[:], ACTF.Exp,
                                                     accum_out=denom[:])
                                rden = sbp.tile([128, 1], F32, tag="rden")
                                nc.vector.reciprocal(rden[:], denom[:])
                                Pn = sbp.tile([128, Wb], F32R, tag="Pn")
                                nc.vector.tensor_scalar_mul(Pn[:], P_t[:], rden[:])
                                for lj in range(NJ):
                                    jc = jst // 128 + lj
                                    idx = next(i for i, (c, _, _) in enumerate(ginfo)
                                               if c == jc)
                                    ps_t = psT.tile([128, 128], F32R, tag="ptps")
                                    nc.tensor.transpose(
                                        ps_t[:], Pn[:, lj * 128:(lj + 1) * 128], ident[:])
                                    nc.vector.tensor_copy(
                                        pts[h][idx][:, half * 128:(half + 1) * 128],
                                        ps_t[:].bitcast(F32))
                        # PV for this group
                        for h in range(2):
                            for i, (jc, inA, inB) in enumerate(ginfo):
                                if not inA:
                                    nc.vector.tensor_copy(pts[h][i][:, 0:128], zero_r[:])
                                if not inB:
                                    nc.vector.tensor_copy(pts[h][i][:, 128:256], zero_r[:])
                            for cc in range(2):
                                eps = psE.tile([128, 256], F32, tag="eps")
                                for i, (jc, _, _) in enumerate(ginfo):
                                    nc.tensor.matmul(
                                        eps[:], vt[jc][:, cc * 128:(cc + 1) * 128],
                                        pts[h][i][:],
                                        start=(i == 0), stop=(i == len(ginfo) - 1),
                                    )
                                nc.scalar.activation(
                                    encT[2 * h + cc][:, g * 256:(g + 1) * 256],
                                    eps[:], ACTF.Copy)
                if debug_taps:
                    for c in range(4):
                        nc.sync.dma_start(
                            out=taps["encT_tap"][c * 128:(c + 1) * 128, :],
                            in_=encT[c][:].bitcast(F32))

                # ---------------- stage C: output projection ------------------
                with (
                    tc.tile_pool(name="outp", bufs=2) as outp,
                    tc.tile_pool(name="psO", bufs=2, space="PSUM") as psO,
                ):
                    for tt in range(NT):
                        ops = psO.tile([128, T], F32, tag="ops")
                        for cc in range(4):
                            for nb in range(4):
                                nc.tensor.matmul(
                                    ops[:, nb * 512:(nb + 1) * 512],
                                    encT[cc][:, tt * 128:(tt + 1) * 128],
                                    wout_t[cc][:, nb * 512:(nb + 1) * 512],
                                    start=(cc == 0), stop=(cc == 3),
                                )
                        ob = outp.tile([128, T], F32, tag="ob")
                        nc.scalar.activation(ob[:], ops[:], ACTF.Copy)
                        nc.sync.dma_start(
                            out=yp[tt * 128:(tt + 1) * 128, :], in_=ob[:])

    nc.compile()
    return nc


def kernel(x, positions, attn_mask, wq, wkv, wout, q_scale, k_scale):
    x = np.ascontiguousarray(x, np.float32)
    positions = np.asarray(positions)
    wq = np.ascontiguousarray(wq, np.float32)
    wkv = np.ascontiguousarray(wkv, np.float32)
    wout = np.ascontiguousarray(wout, np.float32)
    q_scale = np.asarray(q_scale, np.float32)
    k_scale = np.asarray(k_scale, np.float32)

    valid, Wb, js = _geometry(positions, attn_mask)
    shared = not (q_scale.any() or k_scale.any())

    key = (Wb, js, shared, DEBUG_TAPS)
    if key not in _prog_cache:
        _prog_cache[key] = _build(Wb, js, shared, DEBUG_TAPS)
    nc = _prog_cache[key]

    # host-side bias bands: 0 where valid, MASK_NEG elsewhere (incl. padding)
    bias = np.full((B, NT, 128, Wb), MASK_NEG, np.float32)
    for it in range(NT):
        j0 = js[it]
        w = min(Wb, T - j0)
        vslab = valid[:, it * 128:(it + 1) * 128, j0:j0 + w]
        bias[:, it, :, :w][vslab] = 0.0
    bias = bias.astype(ml_dtypes.bfloat16)

    ident = np.eye(128, dtype=np.float32)
    ones1 = np.ones((1, 128), np.float32)
    onesc = np.ones((128, 1), np.float32)

    in_maps = []
    for core in range(8):
        b, kh = divmod(core, NUM_KV_HEADS)
        m = {
            "xT": np.ascontiguousarray(x[b].T),
            "wq": np.ascontiguousarray(wq[:, kh * 512:(kh + 1) * 512]),
            "wk": np.ascontiguousarray(wkv[:, kh * 256:(kh + 1) * 256]),
            "wv": np.ascontiguousarray(wkv[:, 1024 + kh * 256: 1024 + (kh + 1) * 256]),
            "wout": np.ascontiguousarray(wout[kh * 512:(kh + 1) * 512, :]),
            "ident": ident, "ones1": ones1, "onesc": onesc,
            "bias": bias[b],
        }
        if shared:
            ct, st, _, _ = _rope_tables(positions[b], np.zeros(HEAD_DIM, np.float32))
            m["ct"], m["st"] = ct, st
        else:
            for nm, tb in zip(("cq1", "sq1", "cq2", "sq2"),
                              _rope_tables(positions[b], q_scale)):
                m[nm] = tb
            for nm, tb in zip(("ck1", "sk1", "ck2", "sk2"),
                              _rope_tables(positions[b], k_scale)):
                m[nm] = tb
        in_maps.append(m)

    res = run_bass_kernel_spmd(nc, in_maps, list(range(8)))
    kernel._last_results = res
    out = np.empty((B, T, T), np.float32)
    for b in range(B):
        acc = res.results[b * NUM_KV_HEADS]["yp"].astype(np.float64)
        for kh in range(1, NUM_KV_HEADS):
            acc += res.results[b * NUM_KV_HEADS + kh]["yp"]
        out[b] = acc.astype(np.float32)
    return out
